# revision 1
# baseline (speedup 1.0000x reference)
import sys
from contextlib import ExitStack

import numpy as np

sys.path.insert(0, "/opt/trn_rl_repo")

import ml_dtypes

BF16 = ml_dtypes.bfloat16

# Problem constants (hardcoded per contract)
N_NODES = 50000
N_EDGES = 1600000
G = 32        # EDGE_FEAT
GP = G + 1    # feat cols + ones column (for segment sum S / b_e fold)
HID = 64      # EDGE_HIDDEN
H = 128       # NODE_FEAT
CORES = 8
NT = 49       # tiles (of 128 nodes) per core
NPC = NT * 128
NPAD = NPC * CORES
TW = GP * H           # 4224 pf cols per tile block
CW = TW + H           # 4352 = pf block + logits block
NEG = -10000.0        # padding logit -> exp == 0

GROUP_SIZES = [4] * 12 + [1]


def _pairs(T):
    if T == 1:
        return [(0,)]
    return [tuple(range(i, i + 2)) for i in range(0, T, 2)]


def _build_bass(gspec):
    """gspec: tuple of (K, T) per group; sum T == NT."""
    from concourse import bacc, mybir
    import concourse.tile as tile
    import concourse.bass_interp as _bi
    from concourse.cost_model import InstructionCostModel, as_legacy_model
    from concourse.hw_specs import get_hw_spec

    # The tile scheduler's CoreSim prices DMA with the legacy v1 model
    # (~0.4 ns/B), 100x slower than the timeline model's 360 B/ns bus. That
    # skew makes it order next-group compute behind whole tails. Feed the
    # scheduler v2 costs via the on_inst_cost hook while building.
    class _V2CoreSim(_bi.CoreSim):
        def __init__(self, *a, **k):
            super().__init__(*a, **k)
            cm = InstructionCostModel(get_hw_spec(self.module.trn_type))
            import os
            dbg = os.environ.get("V2DBG")
            stats = [0, 0]
            def _cb(inst, d0, c0):
                try:
                    r = as_legacy_model(cm.visit(inst, self))
                    stats[0] += 1
                    return r
                except Exception:
                    stats[1] += 1
                    return (d0, c0)
            self._sim_state.on_inst_cost = _cb
            if dbg:
                import atexit
                atexit.register(lambda: print("V2CB ok/fail:", stats))

    f32 = mybir.dt.float32
    bf = mybir.dt.bfloat16
    AF = mybir.ActivationFunctionType
    OP = mybir.AluOpType

    nc_obj = bacc.Bacc(
        "TRN2", target_bir_lowering=False, debug=False,
        enable_asserts=False, num_devices=CORES,
    )

    TOTROWS = sum(K * T for K, T in gspec)
    pf_d = nc_obj.dram_tensor("pf", [TOTROWS, CW], bf, kind="ExternalInput").ap()
    nfT_d = nc_obj.dram_tensor("nfT", [H, NPC], bf, kind="ExternalInput").ap()
    nf_d = nc_obj.dram_tensor("nf", [H, NT * H], bf, kind="ExternalInput").ap()
    weTa_d = nc_obj.dram_tensor("weTa", [GP, HID], bf, kind="ExternalInput").ap()
    wih_d = nc_obj.dram_tensor("wih", [HID + 1, 3 * H], bf, kind="ExternalInput").ap()
    whh_d = nc_obj.dram_tensor("whh", [H, 3 * H], bf, kind="ExternalInput").ap()
    bhhn_d = nc_obj.dram_tensor("bhhn", [1, H], bf, kind="ExternalInput").ap()
    onesk_d = nc_obj.dram_tensor("onesk", [H, 1], bf, kind="ExternalInput").ap()
    ones1h_d = nc_obj.dram_tensor("ones1h", [1, H], bf, kind="ExternalInput").ap()
    identb_d = nc_obj.dram_tensor("identb", [H, H], bf, kind="ExternalInput").ap()
    hout_d = nc_obj.dram_tensor("hout", [H, NT * H], bf, kind="ExternalOutput").ap()

    _orig_coresim = tile.CoreSim
    tile.CoreSim = _V2CoreSim
    try:
        _build_body(nc_obj, gspec, tile, mybir,
                    pf_d, nfT_d, nf_d, weTa_d, wih_d, whh_d, bhhn_d,
                    onesk_d, ones1h_d, identb_d, hout_d)
    finally:
        tile.CoreSim = _orig_coresim

    nc_obj.compile()
    return nc_obj


def _build_body(nc_obj, gspec, tile, mybir,
                pf_d, nfT_d, nf_d, weTa_d, wih_d, whh_d, bhhn_d,
                onesk_d, ones1h_d, identb_d, hout_d):
    f32 = mybir.dt.float32
    bf = mybir.dt.bfloat16
    AF = mybir.ActivationFunctionType
    OP = mybir.AluOpType

    with tile.TileContext(nc_obj) as tc, ExitStack() as ctx:
        nc = tc.nc
        cpool = ctx.enter_context(tc.tile_pool(name="consts", bufs=1))
        weTa = cpool.tile([GP, HID], bf, tag="weTa")
        nc.sync.dma_start(weTa[:], weTa_d)
        wih = cpool.tile([HID + 1, 3 * H], bf, tag="wih")
        nc.sync.dma_start(wih[:], wih_d)
        whh = cpool.tile([H, 3 * H], bf, tag="whh")
        nc.sync.dma_start(whh[:], whh_d)
        bhhn = cpool.tile([1, H], bf, tag="bhhn")
        nc.sync.dma_start(bhhn[:], bhhn_d)
        onesk = cpool.tile([H, 1], bf, tag="onesk")
        nc.sync.dma_start(onesk[:], onesk_d)
        ones1h = cpool.tile([1, H], bf, tag="ones1h")
        nc.sync.dma_start(ones1h[:], ones1h_d)
        identb = cpool.tile([H, H], bf, tag="identb")
        nc.sync.dma_start(identb[:], identb_d)
        # whole-core node features (h-major for matmuls, partition-major for
        # the elementwise tail)
        nfT_all = cpool.tile([H, NPC], bf, tag="nfT_all")
        nc.sync.dma_start(nfT_all[:], nfT_d)
        nf_all = cpool.tile([H, NT * H], bf, tag="nf_all")
        nc.sync.dma_start(nf_all[:], nf_d)

        inp = ctx.enter_context(tc.tile_pool(name="inp", bufs=2))
        mid = ctx.enter_context(tc.tile_pool(name="mid", bufs=4))
        pY = ctx.enter_context(tc.tile_pool(name="pY", bufs=1, space="PSUM"))
        pS = ctx.enter_context(tc.tile_pool(name="pS", bufs=1, space="PSUM"))
        pC = ctx.enter_context(tc.tile_pool(name="pC", bufs=1, space="PSUM"))
        pX = ctx.enter_context(tc.tile_pool(name="pX", bufs=1, space="PSUM"))
        pR = ctx.enter_context(tc.tile_pool(name="pR", bufs=2, space="PSUM"))
        pN = ctx.enter_context(tc.tile_pool(name="pN", bufs=2, space="PSUM"))
        outp = ctx.enter_context(tc.tile_pool(name="outp", bufs=2))

        def front(K, T, tix, rof):
            c0 = tix * H
            TH = T * H
            # pf/exp in half-group chunks to shorten the per-group chain
            ex = mid.tile([K, TH], bf, tag="ex")
            pf_halves = []
            for t0 in range(T):
                pf_h = inp.tile([K, CW], bf, tag=f"pf{t0}")
                nc.sync.dma_start(
                    pf_h[:],
                    pf_d[rof:rof + K * T, :]
                    .rearrange("(k t) c -> k t c", t=T)[:, t0, :],
                )
                nc.scalar.activation(
                    ex[:].rearrange("k (t h) -> k t h", h=H)[:, t0, :],
                    pf_h[:, TW:],
                    AF.Exp,
                )
                pf_halves.append(pf_h)
            def pf3v(t):
                return pf_halves[t][:, :TW].rearrange("k (p g) -> k p g", g=GP)
            ex3 = ex[:].rearrange("k (t h) -> k t h", h=H)

            # --- aggregation: y[:, n] = pf_n^T @ ex_n ; S per node ---
            y_ps = pY.tile([GP, TH], f32, tag="y")
            s_ps = pS.tile([H, T], f32, tag="s")
            for t in range(T):
                pf3t = pf3v(t)
                for n in range(H):
                    nc.tensor.matmul(
                        y_ps[:, t * H + n: t * H + n + 1],
                        pf3t[:, n, :],
                        ex3[:, t, n:n + 1],
                        start=True, stop=True,
                    )
                nc.tensor.matmul(
                    s_ps[:, t:t + 1], ex3[:, t, :], onesk[0:K, :],
                    start=True, stop=True,
                )
            y_sb = mid.tile([GP, TH], bf, tag="ysb")
            nc.vector.tensor_copy(y_sb[:], y_ps[:])
            rS = mid.tile([H, T], f32, tag="rS")
            nc.vector.reciprocal(rS[:], s_ps[:])
            return dict(K=K, T=T, tix=tix, c0=c0, TH=TH, y_sb=y_sb, rS=rS)

        def tail(st):
            T, tix, c0, TH = st["T"], st["tix"], st["c0"], st["TH"]
            y_sb, rS = st["y_sb"], st["rS"]
            # c_raw[node, :] = y_n^T @ [W_e^T; b_e]  (+= S*b_e via ones col)
            cr_ps = pC.tile([H, T * HID], f32, tag="cr")
            for t in range(T):
                nc.tensor.matmul(
                    cr_ps[:, t * HID:(t + 1) * HID],
                    y_sb[:, t * H:(t + 1) * H], weTa[:],
                    start=True, stop=True,
                )
            # x = c_raw / S + b_e   (node-major)
            x = mid.tile([H, T * HID], bf, tag="x")
            rSb = (
                rS[:].rearrange("p (t o) -> p t o", o=1)
                .broadcast_to([H, T, HID])
            )
            nc.vector.tensor_tensor(
                x[:].rearrange("p (t c) -> p t c", c=HID),
                cr_ps[:].rearrange("p (t c) -> p t c", c=HID),
                rSb, op=OP.mult,
            )
            # ctx' = elu(x)+1 = relu(x) + min(exp(x), 1)
            rn = mid.tile([H, T * HID], bf, tag="rn")
            nc.gpsimd.tensor_scalar_max(rn[:], x[:], 0.0)
            ev = mid.tile([H, T * HID], bf, tag="ev")
            nc.scalar.activation(ev[:], x[:], AF.Exp)
            ctxn = mid.tile([H, T * HID], bf, tag="ctxn")
            nc.vector.scalar_tensor_tensor(
                ctxn[:], ev[:], 1.0, rn[:], OP.min, OP.add,
            )

            # transpose ctx to [HID, node] + ones row
            ctxT_ps = pX.tile([HID, TH], bf, tag="ctxTp")
            for t in range(T):
                nc.tensor.transpose(
                    ctxT_ps[:, t * H:(t + 1) * H],
                    ctxn[:, t * HID:(t + 1) * HID],
                    identb[:],
                )
            ctxT = mid.tile([HID + 1, TH], bf, tag="ctxT")
            nc.vector.tensor_copy(ctxT[0:HID, :], ctxT_ps[:])
            nc.gpsimd.memset(ctxT[HID:HID + 1, :], 1.0)

            # --- GRU gates ---
            t2 = mid.tile([H, TH], f32, tag="t2")
            sigq = mid.tile([H, 2 * TH], bf, tag="sigq")
            for pair in _pairs(T):
                P = len(pair)
                p0 = pair[0]
                rz_ps = pR.tile([H, P * 2 * H], f32, tag="rz")
                ninh_ps = pN.tile([H, P * 2 * H], f32, tag="ninh")
                for ti, t in enumerate(pair):
                    ctx_t = ctxT[:, t * H:(t + 1) * H]
                    nfT_c = nfT_all[:, c0 + t * H:c0 + (t + 1) * H]
                    nc.tensor.matmul(
                        rz_ps[:, ti * 2 * H:(ti + 1) * 2 * H],
                        ctx_t, wih[:, 0:2 * H], start=True, stop=False,
                    )
                    nc.tensor.matmul(
                        rz_ps[:, ti * 2 * H:(ti + 1) * 2 * H],
                        nfT_c, whh[:, 0:2 * H], start=False, stop=True,
                    )
                    nc.tensor.matmul(
                        ninh_ps[:, ti * H:(ti + 1) * H],
                        ctx_t, wih[:, 2 * H:], start=True, stop=False,
                    )
                    nc.tensor.matmul(
                        ninh_ps[:, ti * H:(ti + 1) * H],
                        nfT_c, whh[:, 2 * H:], start=False, stop=True,
                    )
                    nc.tensor.matmul(
                        ninh_ps[:, P * H + ti * H:P * H + (ti + 1) * H],
                        nfT_c, whh[:, 2 * H:], start=True, stop=False,
                    )
                    nc.tensor.matmul(
                        ninh_ps[:, P * H + ti * H:P * H + (ti + 1) * H],
                        ones1h[:], bhhn[:], start=False, stop=True,
                    )
                # trz = tanh(0.5 * rz); r = (1+tr)/2, z = (1+tz)/2
                sig = sigq[:, p0 * 2 * H:(p0 + P) * 2 * H]
                nc.scalar.activation(sig, rz_ps[:], AF.Tanh, scale=0.5)
                sigv = sig.rearrange("p (t two h) -> p t two h", two=2, h=H)
                tr_v = sigv[:, :, 0, :]
                nh_v = ninh_ps[:, P * H:].rearrange("p (t h) -> p t h", h=H)
                ni_v = ninh_ps[:, 0:P * H].rearrange("p (t h) -> p t h", h=H)
                t1 = mid.tile([H, P * H], f32, tag="t1")
                nc.vector.tensor_tensor(
                    t1[:].rearrange("p (t h) -> p t h", h=H),
                    nh_v, tr_v, op=OP.mult,
                )
                nc.vector.tensor_tensor(
                    t2[:, p0 * H:(p0 + P) * H].rearrange("p (t h) -> p t h", h=H),
                    t1[:].rearrange("p (t h) -> p t h", h=H),
                    ni_v, op=OP.add,
                )
            # n = tanh(t2) ; h = 0.5*(n + nf + tz*(nf-n)) ; out = relu
            n_t = mid.tile([H, TH], bf, tag="n")
            nc.scalar.activation(n_t[:], t2[:], AF.Tanh)
            nf_v = nf_all[:, c0:c0 + TH]
            d_t = mid.tile([H, TH], bf, tag="d")
            nc.vector.tensor_tensor(d_t[:], nf_v, n_t[:], op=OP.subtract)
            tz_q = sigq[:].rearrange("p (t two h) -> p t two h", two=2, h=H)[:, :, 1, :]
            zd = mid.tile([H, TH], bf, tag="zd")
            nc.vector.tensor_tensor(
                zd[:].rearrange("p (t h) -> p t h", h=H),
                tz_q,
                d_t[:].rearrange("p (t h) -> p t h", h=H),
                op=OP.mult,
            )
            s1 = mid.tile([H, TH], bf, tag="s1")
            nc.vector.tensor_tensor(s1[:], n_t[:], nf_v, op=OP.add)
            hp = mid.tile([H, TH], bf, tag="hp")
            nc.vector.tensor_tensor(hp[:], s1[:], zd[:], op=OP.add)
            ho = outp.tile([H, TH], bf, tag="ho")
            nc.vector.tensor_scalar(
                ho[:], hp[:], 0.5, 0.0, op0=OP.mult, op1=OP.max,
            )
            nc.sync.dma_start(hout_d[:, c0:c0 + TH], ho[:])

        # software pipeline: tail(g-1) emitted before front(g) so each
        # engine's in-order queue interleaves two groups
        rof = 0
        tix = 0
        pend = None
        for (K, T) in gspec:
            with tc.high_priority(offset=1200):
                f = front(K, T, tix, rof)
            if pend is not None:
                tail(pend)
            pend = f
            rof += K * T
            tix += T
        tail(pend)
        assert tix == NT


_NC_CACHE = None
_NC_GSPEC = None


def _prep(inputs):
    el = np.ascontiguousarray(np.asarray(inputs["edge_logits"], np.float32)[:, 0])
    ef = np.ascontiguousarray(np.asarray(inputs["edge_feats"], np.float32))
    nf = np.asarray(inputs["node_feats"], np.float32)
    dst = np.asarray(inputs["dst"]).astype(np.int64)
    W_e = np.asarray(inputs["W_e"], np.float32)
    b_e = np.asarray(inputs["b_e"], np.float32)
    W_ih = np.asarray(inputs["W_ih"], np.float32)
    W_hh = np.asarray(inputs["W_hh"], np.float32)
    b_ih = np.asarray(inputs["b_ih"], np.float32)
    b_hh = np.asarray(inputs["b_hh"], np.float32)

    deg = np.bincount(dst, minlength=N_NODES)
    assert deg.max() <= 128, f"max in-degree {deg.max()} > 128"
    perm = np.argsort(-deg, kind="stable")
    perm_full = np.concatenate([perm, np.arange(N_NODES, NPAD)])
    deg_sorted = np.zeros(NPAD, np.int64)
    deg_sorted[:N_NODES] = deg[perm]
    inv = np.empty(NPAD, np.int64)
    inv[perm_full] = np.arange(NPAD)

    gspec = []
    r = 0
    for Tsz in GROUP_SIZES:
        lo = r * 8 * 128
        hi = (r + Tsz) * 8 * 128
        K = max(1, int(deg_sorted[lo:hi].max()))
        gspec.append((K, Tsz))
        r += Tsz
    assert r == NT
    gspec = tuple(gspec)

    TOTROWS = sum(K * T for K, T in gspec)
    rank_group = np.zeros(NT, np.int64)
    rank_t = np.zeros(NT, np.int64)
    grow = np.zeros(len(gspec), np.int64)
    Ts = np.array([T for _, T in gspec], np.int64)
    acc = 0
    ri = 0
    for gi, (K, T) in enumerate(gspec):
        grow[gi] = acc
        for t in range(T):
            rank_group[ri + t] = gi
            rank_t[ri + t] = t
        acc += K * T
        ri += T

    # --- edge scatter into per-core slot-major blocks ---
    order = np.argsort(dst, kind="stable")
    sd = dst[order]
    starts = np.zeros(N_NODES + 1, np.int64)
    np.cumsum(deg, out=starts[1:])
    jrank = np.arange(N_EDGES, dtype=np.int64) - starts[sd]
    pos = inv[sd]
    gt = pos // 128
    p = pos % 128
    core = gt % 8
    rank = gt // 8
    gi_e = rank_group[rank]
    row = grow[gi_e] + jrank * Ts[gi_e] + rank_t[rank]

    pf = np.zeros((CORES, TOTROWS, CW), BF16)
    pf[:, :, G:TW:GP] = 1.0
    pf[:, :, TW:] = NEG
    flat = pf.reshape(-1)
    base = (core * TOTROWS + row) * CW
    colf = p * GP
    idx2 = base[:, None] + colf[:, None] + np.arange(G)[None, :]
    flat[idx2.reshape(-1)] = ef[order].astype(BF16).reshape(-1)
    flat[base + TW + p] = el[order].astype(BF16)

    # --- node features (permuted, per core) ---
    nf_sorted = np.zeros((NPAD, H), np.float32)
    nf_sorted[:N_NODES] = nf[perm]
    core_idx = (np.arange(NT)[:, None] * 8 + np.arange(CORES)[None, :]) * 128
    idx_c = core_idx.T[:, :, None] + np.arange(128)[None, None, :]
    idx_c = idx_c.reshape(CORES, NPC)
    nf_c = nf_sorted[idx_c]                              # [CORES, NPC, H]
    nfT_c = np.ascontiguousarray(
        np.transpose(nf_c, (0, 2, 1))
    ).astype(BF16)                                       # [CORES, H, NPC]
    # partition-major: nf_pm[c, p, t*H+h] = nf_c[c, t*128+p, h]
    nf_pm = np.ascontiguousarray(
        np.transpose(nf_c.reshape(CORES, NT, 128, H), (0, 2, 1, 3))
        .reshape(CORES, 128, NT * H)
    ).astype(BF16)

    colsum = W_ih.sum(axis=1)
    wih_bias = b_ih - colsum
    wih_bias[:2 * H] += b_hh[:2 * H]
    wih_bias[2 * H:] += 0.5 * b_hh[2 * H:]
    whh_mod = np.ascontiguousarray(W_hh.T).copy()
    whh_mod[:, 2 * H:] *= 0.5

    common = {
        "weTa": np.vstack([W_e.T, b_e[None, :]]).astype(BF16),
        "wih": np.vstack([W_ih.T, wih_bias[None, :]]).astype(BF16),
        "whh": whh_mod.astype(BF16),
        "bhhn": (0.5 * b_hh[None, 2 * H:]).astype(BF16),
        "onesk": np.ones((H, 1), BF16),
        "ones1h": np.ones((1, H), BF16),
        "identb": np.eye(H, dtype=np.float32).astype(BF16),
    }
    in_maps = [
        dict(pf=pf[c], nfT=nfT_c[c], nf=nf_pm[c], **common)
        for c in range(CORES)
    ]
    return gspec, in_maps, idx_c, inv


def kernel(**inputs):
    global _NC_CACHE, _NC_GSPEC
    from concourse.bass_utils import run_bass_kernel_spmd

    gspec, in_maps, idx_c, inv = _prep(inputs)
    if _NC_CACHE is None or _NC_GSPEC != gspec:
        _NC_CACHE = _build_bass(gspec)
        _NC_GSPEC = gspec
    nc = _NC_CACHE
    res = run_bass_kernel_spmd(nc, in_maps, core_ids=list(range(CORES)))
    out_sorted = np.empty((NPAD, H), np.float32)
    for c in range(CORES):
        hout = np.asarray(res.results[c]["hout"], np.float32)  # [128, NT*H]
        out_sorted[idx_c[c]] = (
            hout.reshape(128, NT, H).transpose(1, 0, 2).reshape(NPC, H)
        )
    return out_sorted[inv[:N_NODES]]



# revision 12
# speedup vs baseline: 1.0267x; 1.0267x over previous
import sys
from contextlib import ExitStack

import numpy as np

sys.path.insert(0, "/opt/trn_rl_repo")

import ml_dtypes

BF16 = ml_dtypes.bfloat16

# Problem constants (hardcoded per contract)
N_NODES = 50000
N_EDGES = 1600000
G = 32        # EDGE_FEAT
GP = G + 1    # feat cols + ones column (for segment sum S / b_e fold)
HID = 64      # EDGE_HIDDEN
H = 128       # NODE_FEAT
CORES = 8
NT = 49       # tiles (of 128 nodes) per core
NPC = NT * 128
NPAD = NPC * CORES
NEG = -10000.0  # padding logit -> exp == 0

GROUP_SIZES = [4] * 12 + [1]


def _pick_p(K):
    return max(1, min(128 // K, 8))


def _chunks(P):
    nfull, rem = 128 // P, 128 % P
    ch = [(i * P, P) for i in range(nfull)]
    if rem:
        ch.append((nfull * P, rem))
    return ch


def _pairs(T):
    if T == 1:
        return [(0,)]
    return [tuple(range(i, i + 2)) for i in range(0, T, 2)]


def _emit_order(n):
    # smallest group (last in gspec) first to shorten pipeline fill
    return [n - 1] + list(range(n - 1))


def _build_bass(gspec):
    """gspec: tuple of (K, T) per group; sum T == NT."""
    from concourse import bacc, mybir
    import concourse.tile as tile
    import concourse.bass_interp as _bi
    from concourse.cost_model import InstructionCostModel, as_legacy_model
    from concourse.hw_specs import get_hw_spec

    # The tile scheduler's CoreSim prices DMA with the legacy v1 model,
    # far off the timeline model's bus rate. Feed the scheduler v2 costs
    # via the on_inst_cost hook while building (restored after).
    class _V2CoreSim(_bi.CoreSim):
        def __init__(self, *a, **k):
            super().__init__(*a, **k)
            cm = InstructionCostModel(get_hw_spec(self.module.trn_type))

            def _cb(inst, d0, c0):
                try:
                    return as_legacy_model(cm.visit(inst, self))
                except Exception:
                    return (d0, c0)

            self._sim_state.on_inst_cost = _cb

    bf = mybir.dt.bfloat16

    nc_obj = bacc.Bacc(
        "TRN2", target_bir_lowering=False, debug=False,
        enable_asserts=False, num_devices=CORES,
    )

    pf_ds = []
    for gi, (K, T) in enumerate(gspec):
        P = _pick_p(K)
        C = len(_chunks(P))
        PK = P * K
        W = T * C * GP + T * 128
        pf_ds.append(
            nc_obj.dram_tensor(f"pf{gi}", [PK, W], bf, kind="ExternalInput").ap()
        )
    nfT_d = nc_obj.dram_tensor("nfT", [H, NPC], bf, kind="ExternalInput").ap()
    nf_d = nc_obj.dram_tensor("nf", [H, NT * H], bf, kind="ExternalInput").ap()
    consts_d = nc_obj.dram_tensor("consts", [128, 1088], bf, kind="ExternalInput").ap()
    hout_d = nc_obj.dram_tensor("hout", [H, NT * H], bf, kind="ExternalOutput").ap()

    _orig_coresim = tile.CoreSim
    tile.CoreSim = _V2CoreSim
    try:
        _build_body(nc_obj, gspec, tile, mybir, pf_ds, nfT_d, nf_d, consts_d, hout_d)
    finally:
        tile.CoreSim = _orig_coresim

    nc_obj.compile()
    return nc_obj


def _build_body(nc_obj, gspec, tile, mybir, pf_ds, nfT_d, nf_d, consts_d, hout_d):
    f32 = mybir.dt.float32
    bf = mybir.dt.bfloat16
    AF = mybir.ActivationFunctionType
    OP = mybir.AluOpType

    order = _emit_order(len(gspec))
    # emission-order tile offsets (host lays out nfT/nf/hout in this order)
    emit_off = {}
    acc = 0
    for gi in order:
        emit_off[gi] = acc
        acc += gspec[gi][1]
    assert acc == NT

    with tile.TileContext(nc_obj) as tc, ExitStack() as ctx:
        nc = tc.nc
        cpool = ctx.enter_context(tc.tile_pool(name="consts", bufs=1))
        consts = cpool.tile([128, 1088], bf, tag="consts")
        nc.sync.dma_start(consts[:], consts_d)
        weTa = consts[0:GP, 0:64]
        wih = consts[0:HID + 1, 64:448]
        whh = consts[:, 448:832]
        bhhn = consts[HID:HID + 1, 832:960]
        identb = consts[:, 960:1088]
        one11 = consts[0:1, 960:961]  # identb[0,0] == 1.0 viewed as [1,1]

        # whole-core node features (h-major for matmuls, partition-major for
        # the elementwise tail)
        nfT_all = cpool.tile([H, NPC], bf, tag="nfT_all")
        nf_all = cpool.tile([H, NT * H], bf, tag="nf_all")
        # output accumulator (stored by two DMAs near the end)
        ho_all = cpool.tile([H, NT * H], bf, tag="ho_all")
        # persistent ctxT buffers with a constant ones row (row HID)
        ctxTbufs = [
            cpool.tile([HID + 1, 512], bf, tag=f"ctxT{i}", name=f"ctxT{i}")
            for i in range(2)
        ]
        for t_ in ctxTbufs:
            nc.gpsimd.memset(t_[HID:HID + 1, :], 1.0)

        inp = ctx.enter_context(tc.tile_pool(name="inp", bufs=2))
        mid = ctx.enter_context(tc.tile_pool(name="mid", bufs=2))
        pY = ctx.enter_context(tc.tile_pool(name="pY", bufs=1, space="PSUM"))
        pC = ctx.enter_context(tc.tile_pool(name="pC", bufs=1, space="PSUM"))
        pX = ctx.enter_context(tc.tile_pool(name="pX", bufs=2, space="PSUM"))
        pR = ctx.enter_context(tc.tile_pool(name="pR", bufs=1, space="PSUM"))
        pN = ctx.enter_context(tc.tile_pool(name="pN", bufs=2, space="PSUM"))

        def stageA(gi):
            K, T = gspec[gi]
            P = _pick_p(K)
            ch = _chunks(P)
            C = len(ch)
            PK = P * K
            W = T * C * GP + T * 128
            pfg = inp.tile([128, W], bf, tag="pfg")
            nc.sync.dma_start(pfg[0:PK, :], pf_ds[gi])
            return dict(gi=gi, K=K, T=T, P=P, ch=ch, C=C, PK=PK, W=W, pfg=pfg)

        def stageB(st):
            K, T, ch, C, PK = st["K"], st["T"], st["ch"], st["C"], st["PK"]
            pfg = st["pfg"]
            LOG0 = T * C * GP
            ex = mid.tile([128, T * 128], bf, tag="ex")
            nc.scalar.activation(ex[0:PK, :], pfg[0:PK, LOG0:], AF.Exp)
            y_ps = pY.tile([GP, T * 128], f32, tag="y")
            for t in range(T):
                for c, (cs, pc) in enumerate(ch):
                    nc.tensor.matmul(
                        y_ps[:, t * 128 + cs: t * 128 + cs + pc],
                        pfg[0:pc * K, t * C * GP + c * GP: t * C * GP + (c + 1) * GP],
                        ex[0:pc * K, t * 128 + cs: t * 128 + cs + pc],
                        start=True, stop=True,
                    )
            y_sb = mid.tile([GP, T * 128], bf, tag="ysb")
            nc.gpsimd.tensor_copy(y_sb[:], y_ps[:])
            # S (row 32 of y) -> node-per-partition via PE transpose
            sT_ps = pX.tile([H, T], bf, tag="sT", bufs=1)
            for t in range(T):
                nc.tensor.matmul(
                    sT_ps[:, t:t + 1],
                    y_sb[0:1, t * 128:(t + 1) * 128],
                    one11,
                    is_transpose=True,
                )
            rS = mid.tile([H, T], f32, tag="rS")
            nc.vector.reciprocal(rS[:], sT_ps[:])
            st.update(y_sb=y_sb, rS=rS)
            return st

        def stageC(st):
            gi, T = st["gi"], st["T"]
            y_sb, rS = st["y_sb"], st["rS"]
            c0 = emit_off[gi] * 128
            TH = T * 128
            # c_raw[node, :] = y_t^T @ [W_e^T; b_e]  (+= S*b_e via ones col)
            cr_ps = pC.tile([H, T * HID], f32, tag="cr")
            for t in range(T):
                nc.tensor.matmul(
                    cr_ps[:, t * HID:(t + 1) * HID],
                    y_sb[:, t * 128:(t + 1) * 128], weTa,
                    start=True, stop=True,
                )
            # x = c_raw / S (+ b_e)   (node-major)
            x = mid.tile([H, T * HID], bf, tag="x")
            rSb = (
                rS[:].rearrange("p (t o) -> p t o", o=1)
                .broadcast_to([H, T, HID])
            )
            nc.vector.tensor_tensor(
                x[:].rearrange("p (t c) -> p t c", c=HID),
                cr_ps[:].rearrange("p (t c) -> p t c", c=HID),
                rSb, op=OP.mult,
            )
            # ctx' = elu(x)+1 = relu(x) + min(exp(x), 1)
            ev = mid.tile([H, T * HID], bf, tag="ev")
            nc.scalar.activation(ev[:], x[:], AF.Exp)
            rn = mid.tile([H, T * HID], bf, tag="rn")
            nc.gpsimd.tensor_scalar_max(rn[:], x[:], 0.0)
            ctxn = mid.tile([H, T * HID], bf, tag="ctxn")
            nc.vector.scalar_tensor_tensor(
                ctxn[:], ev[:], 1.0, rn[:], OP.min, OP.add,
            )
            # transpose ctx to [HID, node] (ones row persistent in ctxT buf)
            ctxT_ps = pX.tile([HID, TH], bf, tag="ctxTp")
            for t in range(T):
                nc.tensor.transpose(
                    ctxT_ps[:, t * 128:(t + 1) * 128],
                    ctxn[:, t * HID:(t + 1) * HID],
                    identb,
                )
            ctxT = ctxTbufs[st["buf"]]
            nc.gpsimd.tensor_copy(ctxT[0:HID, 0:TH], ctxT_ps[:])

            # --- GRU gates ---
            t2 = mid.tile([H, TH], bf, tag="t2")
            sigq = mid.tile([H, 2 * TH], bf, tag="sigq")
            for pair in _pairs(T):
                P2 = len(pair)
                p0 = pair[0]
                rz_ps = pR.tile([H, P2 * 2 * H], f32, tag="rz")
                inhn_ps = pN.tile([H, P2 * 2 * H], f32, tag="inhn")
                for ti, t in enumerate(pair):
                    ctx_t = ctxT[:, t * H:(t + 1) * H]
                    nfT_c = nfT_all[:, c0 + t * H:c0 + (t + 1) * H]
                    nc.tensor.matmul(
                        rz_ps[:, ti * 2 * H:(ti + 1) * 2 * H],
                        ctx_t, wih[:, 0:2 * H], start=True, stop=False,
                    )
                    nc.tensor.matmul(
                        rz_ps[:, ti * 2 * H:(ti + 1) * 2 * H],
                        nfT_c, whh[:, 0:2 * H], start=False, stop=True,
                    )
                    # in_n = ctx@wih_n (+bias via ones row of ctxT)
                    nc.tensor.matmul(
                        inhn_ps[:, ti * 2 * H:ti * 2 * H + H],
                        ctx_t, wih[:, 2 * H:], start=True, stop=True,
                    )
                    # hn = 0.5*(nf@whh_n) + 0.5*bhh_n
                    nc.tensor.matmul(
                        inhn_ps[:, ti * 2 * H + H:(ti + 1) * 2 * H],
                        nfT_c, whh[:, 2 * H:], start=True, stop=False,
                    )
                    nc.tensor.matmul(
                        inhn_ps[:, ti * 2 * H + H:(ti + 1) * 2 * H],
                        ctxT[HID:HID + 1, t * H:(t + 1) * H], bhhn,
                        start=False, stop=True,
                    )
                # trz = tanh(0.5 * rz); r = (1+tr)/2, z = (1+tz)/2
                sig = sigq[:, p0 * 2 * H:(p0 + P2) * 2 * H]
                nc.scalar.activation(sig, rz_ps[:], AF.Tanh, scale=0.5)
                sigv = sig.rearrange("p (t two h) -> p t two h", two=2, h=H)
                tr_v = sigv[:, :, 0, :]
                iv = inhn_ps[:].rearrange("p (t two h) -> p t two h", two=2, h=H)
                in_v = iv[:, :, 0, :]
                hn_v = iv[:, :, 1, :]
                # t2 = (tr+1)*hn + in_n  (= i_n + r*(h_n + bhh_n))
                t1 = mid.tile([H, P2 * H], bf, tag="t1")
                nc.vector.scalar_tensor_tensor(
                    t1[:].rearrange("p (t h) -> p t h", h=H),
                    tr_v, 1.0, hn_v, OP.add, OP.mult,
                )
                nc.vector.tensor_tensor(
                    t2[:, p0 * H:(p0 + P2) * H].rearrange("p (t h) -> p t h", h=H),
                    t1[:].rearrange("p (t h) -> p t h", h=H),
                    in_v, op=OP.add,
                )
            # n = tanh(t2); h = relu(0.5*(n + nf + tz*(nf-n)))
            #   = relu(0.5*[(tz+1)*nf - (tz-1)*n])
            n_t = mid.tile([H, TH], bf, tag="n")
            nc.scalar.activation(n_t[:], t2[:], AF.Tanh)
            nf_v = nf_all[:, c0:c0 + TH]  # pre-scaled? no: plain nf
            tz_q = sigq[:].rearrange("p (t two h) -> p t two h", two=2, h=H)[:, :, 1, :]
            u_t = mid.tile([H, TH], bf, tag="u")
            nc.vector.scalar_tensor_tensor(
                u_t[:].rearrange("p (t h) -> p t h", h=H),
                tz_q, 1.0,
                nf_v.rearrange("p (t h) -> p t h", h=H),
                OP.add, OP.mult,
            )
            v_t = mid.tile([H, TH], bf, tag="v")
            nc.vector.scalar_tensor_tensor(
                v_t[:].rearrange("p (t h) -> p t h", h=H),
                tz_q, -1.0, n_t[:].rearrange("p (t h) -> p t h", h=H),
                OP.add, OP.mult,
            )
            hp = mid.tile([H, TH], bf, tag="hp")
            nc.vector.tensor_tensor(hp[:], u_t[:], v_t[:], op=OP.subtract)
            nc.vector.tensor_scalar(
                ho_all[:, c0:c0 + TH], hp[:], 0.5, 0.0, op0=OP.mult, op1=OP.max,
            )

        # --- 3-stage software pipeline over groups ---
        NG = len(gspec)
        stA = stB = None
        bufc = 0
        consumed = 0
        store1_cols = None
        for i in range(NG + 2):
            if i < NG:
                with tc.high_priority(offset=1200):
                    a = stageA(order[i])
            else:
                a = None
            if i == 1:
                # node features load after the first pf group
                nc.sync.dma_start(nfT_all[:], nfT_d)
                nc.sync.dma_start(nf_all[:], nf_d)
            if stB is not None:
                stB["buf"] = bufc % 2
                bufc += 1
                stageC(stB)
                consumed += stB["T"]
            if stA is not None:
                with tc.high_priority(offset=600):
                    stB = stageB(stA)
            else:
                stB = None
            stA = a
            if i == NG and store1_cols is None:
                # first partial store once all but the last group's tail is
                # queued; covers everything already computed
                store1_cols = consumed * 128
                nc.sync.dma_start(
                    hout_d[:, 0:store1_cols], ho_all[:, 0:store1_cols]
                )
        nc.sync.dma_start(
            hout_d[:, store1_cols:], ho_all[:, store1_cols:]
        )
        assert consumed == NT


_NC_CACHE = None
_NC_GSPEC = None


def _prep(inputs):
    el = np.ascontiguousarray(np.asarray(inputs["edge_logits"], np.float32)[:, 0])
    ef = np.ascontiguousarray(np.asarray(inputs["edge_feats"], np.float32))
    nf = np.asarray(inputs["node_feats"], np.float32)
    dst = np.asarray(inputs["dst"]).astype(np.int64)
    W_e = np.asarray(inputs["W_e"], np.float32)
    b_e = np.asarray(inputs["b_e"], np.float32)
    W_ih = np.asarray(inputs["W_ih"], np.float32)
    W_hh = np.asarray(inputs["W_hh"], np.float32)
    b_ih = np.asarray(inputs["b_ih"], np.float32)
    b_hh = np.asarray(inputs["b_hh"], np.float32)

    deg = np.bincount(dst, minlength=N_NODES)
    assert deg.max() <= 128, f"max in-degree {deg.max()} > 128"
    perm = np.argsort(-deg, kind="stable")
    perm_full = np.concatenate([perm, np.arange(N_NODES, NPAD)])
    deg_sorted = np.zeros(NPAD, np.int64)
    deg_sorted[:N_NODES] = deg[perm]
    inv = np.empty(NPAD, np.int64)
    inv[perm_full] = np.arange(NPAD)

    gspec = []
    r = 0
    for Tsz in GROUP_SIZES:
        lo = r * 8 * 128
        hi = (r + Tsz) * 8 * 128
        K = max(1, int(deg_sorted[lo:hi].max()))
        gspec.append((K, Tsz))
        r += Tsz
    assert r == NT
    gspec = tuple(gspec)
    NG = len(gspec)

    # per-group geometry
    K_a = np.array([K for K, _ in gspec], np.int64)
    T_a = np.array([T for _, T in gspec], np.int64)
    P_a = np.array([_pick_p(K) for K, _ in gspec], np.int64)
    C_a = np.array([len(_chunks(int(P))) for P in P_a], np.int64)
    W_a = T_a * C_a * GP + T_a * 128
    PK_a = P_a * K_a

    # rank (tile index in degree order) -> group, t within group
    rank_group = np.zeros(NT, np.int64)
    rank_t = np.zeros(NT, np.int64)
    ri = 0
    for gi, (K, T) in enumerate(gspec):
        for t in range(T):
            rank_group[ri + t] = gi
            rank_t[ri + t] = t
        ri += T

    # --- edge scatter into per-(core, group) blocks ---
    order_e = np.argsort(dst, kind="stable")
    sd = dst[order_e]
    starts = np.zeros(N_NODES + 1, np.int64)
    np.cumsum(deg, out=starts[1:])
    k_e = np.arange(N_EDGES, dtype=np.int64) - starts[sd]
    pos = inv[sd]
    gt = pos // 128
    p_e = pos % 128
    core = gt % 8
    rank = gt // 8
    gi_e = rank_group[rank]
    t_e = rank_t[rank]
    Pe = P_a[gi_e]
    Ke = K_a[gi_e]
    Ce = C_a[gi_e]
    We = W_a[gi_e]
    c_e = p_e // Pe
    j_e = p_e % Pe
    row_e = j_e * Ke + k_e
    fcol_e = t_e * Ce * GP + c_e * GP
    lcol_e = T_a[gi_e] * Ce * GP + t_e * 128 + p_e

    efq = ef[order_e].astype(BF16)
    elq = el[order_e].astype(BF16)

    pf_blocks = []
    for gi in range(NG):
        PK, W, T, C = int(PK_a[gi]), int(W_a[gi]), int(T_a[gi]), int(C_a[gi])
        blk = np.zeros((CORES, PK, W), BF16)
        onescols = (
            np.arange(T)[:, None] * (C * GP) + np.arange(C)[None, :] * GP
        ).reshape(-1)
        blk[:, :, onescols] = 1.0
        blk[:, :, T * C * GP:] = NEG
        pf_blocks.append(blk)

    for gi in range(NG):
        m = gi_e == gi
        blk = pf_blocks[gi]
        W = int(W_a[gi])
        PK = int(PK_a[gi])
        flat = blk.reshape(-1)
        base = (core[m] * PK + row_e[m]) * W
        idx2 = (base + fcol_e[m])[:, None] + 1 + np.arange(G)[None, :]
        flat[idx2.reshape(-1)] = efq[m].reshape(-1)
        flat[base + lcol_e[m]] = elq[m]

    # --- node features (permuted, per core, tiles in EMISSION order) ---
    order_g = _emit_order(NG)
    tile_off = np.zeros(NG, np.int64)
    acc = 0
    for gi, (K, T) in enumerate(gspec):
        tile_off[gi] = acc
        acc += T
    rank_seq = np.concatenate(
        [np.arange(tile_off[gi], tile_off[gi] + gspec[gi][1]) for gi in order_g]
    )  # emission tile index -> rank

    nf_sorted = np.zeros((NPAD, H), np.float32)
    nf_sorted[:N_NODES] = nf[perm]
    core_idx = (rank_seq[:, None] * 8 + np.arange(CORES)[None, :]) * 128
    idx_c = core_idx.T[:, :, None] + np.arange(128)[None, None, :]
    idx_c = idx_c.reshape(CORES, NPC)
    nf_c = nf_sorted[idx_c]                              # [CORES, NPC, H]
    nfT_c = np.ascontiguousarray(
        np.transpose(nf_c, (0, 2, 1))
    ).astype(BF16)                                       # [CORES, H, NPC]
    # partition-major: nf_pm[c, p, t*H+h] = nf_c[c, t*128+p, h]
    nf_pm = np.ascontiguousarray(
        np.transpose(nf_c.reshape(CORES, NT, 128, H), (0, 2, 1, 3))
        .reshape(CORES, 128, NT * H)
    ).astype(BF16)

    colsum = W_ih.sum(axis=1)
    wih_bias = b_ih - colsum
    wih_bias[:2 * H] += b_hh[:2 * H]
    whh_mod = np.ascontiguousarray(W_hh.T).copy()
    whh_mod[:, 2 * H:] *= 0.5

    consts = np.zeros((128, 1088), np.float32)
    consts[0:GP, 0:64] = np.vstack([b_e[None, :], W_e.T])
    consts[0:HID + 1, 64:448] = np.vstack([W_ih.T, wih_bias[None, :]])
    consts[:, 448:832] = whh_mod
    consts[HID:HID + 1, 832:960] = 0.5 * b_hh[None, 2 * H:]
    consts[:, 960:1088] = np.eye(H, dtype=np.float32)

    common = {"consts": consts.astype(BF16)}
    in_maps = []
    for c in range(CORES):
        d = dict(nfT=nfT_c[c], nf=nf_pm[c], **common)
        for gi in range(NG):
            d[f"pf{gi}"] = pf_blocks[gi][c]
        in_maps.append(d)
    return gspec, in_maps, idx_c, inv


def kernel(**inputs):
    global _NC_CACHE, _NC_GSPEC
    from concourse.bass_utils import run_bass_kernel_spmd

    gspec, in_maps, idx_c, inv = _prep(inputs)
    if _NC_CACHE is None or _NC_GSPEC != gspec:
        _NC_CACHE = _build_bass(gspec)
        _NC_GSPEC = gspec
    nc = _NC_CACHE
    res = run_bass_kernel_spmd(nc, in_maps, core_ids=list(range(CORES)))
    out_sorted = np.empty((NPAD, H), np.float32)
    for c in range(CORES):
        hout = np.asarray(res.results[c]["hout"], np.float32)  # [128, NT*H]
        out_sorted[idx_c[c]] = (
            hout.reshape(128, NT, H).transpose(1, 0, 2).reshape(NPC, H)
        )
    return out_sorted[inv[:N_NODES]]


# revision 13
# speedup vs baseline: 1.0533x; 1.0259x over previous
import sys
from contextlib import ExitStack

import numpy as np

sys.path.insert(0, "/opt/trn_rl_repo")

import ml_dtypes

BF16 = ml_dtypes.bfloat16

# Problem constants (hardcoded per contract)
N_NODES = 50000
N_EDGES = 1600000
G = 32        # EDGE_FEAT
GP = G + 1    # feat cols + ones column (for segment sum S / b_e fold)
HID = 64      # EDGE_HIDDEN
H = 128       # NODE_FEAT
CORES = 8
NT = 49       # tiles (of 128 nodes) per core
NPC = NT * 128
NPAD = NPC * CORES
NEG = -10000.0  # padding logit -> exp == 0

GROUP_SIZES = [4] * 12 + [1]


def _pick_p(K):
    return max(1, min(128 // K, 8))


def _chunks(P):
    nfull, rem = 128 // P, 128 % P
    ch = [(i * P, P) for i in range(nfull)]
    if rem:
        ch.append((nfull * P, rem))
    return ch


def _pairs(T):
    if T == 1:
        return [(0,)]
    return [tuple(range(i, i + 2)) for i in range(0, T, 2)]


def _emit_order(n):
    # smallest group (last in gspec) first to shorten pipeline fill
    return [n - 1] + list(range(n - 1))


def _build_bass(gspec):
    """gspec: tuple of (K, T) per group; sum T == NT."""
    from concourse import bacc, mybir
    import concourse.tile as tile
    import concourse.bass_interp as _bi
    from concourse.cost_model import InstructionCostModel, as_legacy_model
    from concourse.hw_specs import get_hw_spec

    # The tile scheduler's CoreSim prices DMA with the legacy v1 model,
    # far off the timeline model's bus rate. Feed the scheduler v2 costs
    # via the on_inst_cost hook while building (restored after).
    class _V2CoreSim(_bi.CoreSim):
        def __init__(self, *a, **k):
            super().__init__(*a, **k)
            cm = InstructionCostModel(get_hw_spec(self.module.trn_type))

            def _cb(inst, d0, c0):
                try:
                    return as_legacy_model(cm.visit(inst, self))
                except Exception:
                    return (d0, c0)

            self._sim_state.on_inst_cost = _cb

    bf = mybir.dt.bfloat16

    nc_obj = bacc.Bacc(
        "TRN2", target_bir_lowering=False, debug=False,
        enable_asserts=False, num_devices=CORES,
    )

    pf_ds = []
    for gi, (K, T) in enumerate(gspec):
        P = _pick_p(K)
        C = len(_chunks(P))
        PK = P * K
        W = T * C * GP + T * 128
        pf_ds.append(
            nc_obj.dram_tensor(f"pf{gi}", [PK, W], bf, kind="ExternalInput").ap()
        )
    nfT_d = nc_obj.dram_tensor("nfT", [H, NPC], bf, kind="ExternalInput").ap()
    nf_d = nc_obj.dram_tensor("nf", [H, NT * H], bf, kind="ExternalInput").ap()
    consts_d = nc_obj.dram_tensor("consts", [128, 1088], bf, kind="ExternalInput").ap()
    hout_d = nc_obj.dram_tensor("hout", [H, NT * H], bf, kind="ExternalOutput").ap()

    _orig_coresim = tile.CoreSim
    tile.CoreSim = _V2CoreSim
    try:
        _build_body(nc_obj, gspec, tile, mybir, pf_ds, nfT_d, nf_d, consts_d, hout_d)
    finally:
        tile.CoreSim = _orig_coresim

    nc_obj.compile()
    return nc_obj


def _build_body(nc_obj, gspec, tile, mybir, pf_ds, nfT_d, nf_d, consts_d, hout_d):
    f32 = mybir.dt.float32
    bf = mybir.dt.bfloat16
    AF = mybir.ActivationFunctionType
    OP = mybir.AluOpType

    order = _emit_order(len(gspec))
    # emission-order tile offsets (host lays out nfT/nf/hout in this order)
    emit_off = {}
    acc = 0
    for gi in order:
        emit_off[gi] = acc
        acc += gspec[gi][1]
    assert acc == NT

    with tile.TileContext(nc_obj) as tc, ExitStack() as ctx:
        nc = tc.nc
        cpool = ctx.enter_context(tc.tile_pool(name="consts", bufs=1))
        consts = cpool.tile([128, 1088], bf, tag="consts")
        nc.sync.dma_start(consts[:], consts_d)
        weTa = consts[0:GP, 0:64]
        wih = consts[0:HID + 1, 64:448]
        whh = consts[:, 448:832]
        bhhn = consts[HID:HID + 1, 832:960]
        identb = consts[:, 960:1088]
        one11 = consts[0:1, 960:961]  # identb[0,0] == 1.0 viewed as [1,1]

        # whole-core node features (h-major for matmuls, partition-major for
        # the elementwise tail)
        nfT_all = cpool.tile([H, NPC], bf, tag="nfT_all")
        nf_all = cpool.tile([H, NT * H], bf, tag="nf_all")
        # output accumulator (stored by two DMAs near the end)
        ho_all = cpool.tile([H, NT * H], bf, tag="ho_all")
        # persistent ctxT buffers with a constant ones row (row HID)
        ctxTbufs = [
            cpool.tile([HID + 1, 512], bf, tag=f"ctxT{i}", name=f"ctxT{i}")
            for i in range(2)
        ]
        for t_ in ctxTbufs:
            nc.gpsimd.memset(t_[HID:HID + 1, :], 1.0)

        inp = ctx.enter_context(tc.tile_pool(name="inp", bufs=2))
        mid = ctx.enter_context(tc.tile_pool(name="mid", bufs=2))
        pY = ctx.enter_context(tc.tile_pool(name="pY", bufs=1, space="PSUM"))
        pC = ctx.enter_context(tc.tile_pool(name="pC", bufs=1, space="PSUM"))
        pX = ctx.enter_context(tc.tile_pool(name="pX", bufs=2, space="PSUM"))
        pR = ctx.enter_context(tc.tile_pool(name="pR", bufs=1, space="PSUM"))
        pN = ctx.enter_context(tc.tile_pool(name="pN", bufs=2, space="PSUM"))

        def stageA(gi):
            K, T = gspec[gi]
            P = _pick_p(K)
            ch = _chunks(P)
            C = len(ch)
            PK = P * K
            W = T * C * GP + T * 128
            pfg = inp.tile([128, W], bf, tag="pfg")
            nc.sync.dma_start(pfg[0:PK, :], pf_ds[gi])
            return dict(gi=gi, K=K, T=T, P=P, ch=ch, C=C, PK=PK, W=W, pfg=pfg)

        def stageB(st):
            K, T, ch, C, PK = st["K"], st["T"], st["ch"], st["C"], st["PK"]
            pfg = st["pfg"]
            LOG0 = T * C * GP
            ex = mid.tile([128, T * 128], bf, tag="ex")
            nc.scalar.activation(ex[0:PK, :], pfg[0:PK, LOG0:], AF.Exp)
            y_ps = pY.tile([GP, T * 128], f32, tag="y")
            for t in range(T):
                for c, (cs, pc) in enumerate(ch):
                    nc.tensor.matmul(
                        y_ps[:, t * 128 + cs: t * 128 + cs + pc],
                        pfg[0:pc * K, t * C * GP + c * GP: t * C * GP + (c + 1) * GP],
                        ex[0:pc * K, t * 128 + cs: t * 128 + cs + pc],
                        start=True, stop=True,
                    )
            y_sb = mid.tile([GP, T * 128], bf, tag="ysb")
            nc.gpsimd.tensor_copy(y_sb[:], y_ps[:])
            # S (row 32 of y) -> node-per-partition via PE transpose
            sT_ps = pX.tile([H, T], bf, tag="sT", bufs=1)
            for t in range(T):
                nc.tensor.matmul(
                    sT_ps[:, t:t + 1],
                    y_sb[0:1, t * 128:(t + 1) * 128],
                    one11,
                    is_transpose=True,
                )
            rS = mid.tile([H, T], f32, tag="rS")
            nc.vector.reciprocal(rS[:], sT_ps[:])
            st.update(y_sb=y_sb, rS=rS)
            return st

        def stageC1(st):
            gi, T = st["gi"], st["T"]
            y_sb, rS = st["y_sb"], st["rS"]
            TH = T * 128
            # c_raw[node, :] = y_t^T @ [W_e^T; b_e]  (+= S*b_e via ones col)
            cr_ps = pC.tile([H, T * HID], f32, tag="cr")
            for t in range(T):
                nc.tensor.matmul(
                    cr_ps[:, t * HID:(t + 1) * HID],
                    y_sb[:, t * 128:(t + 1) * 128], weTa,
                    start=True, stop=True,
                )
            # x = c_raw / S (+ b_e)   (node-major)
            x = mid.tile([H, T * HID], bf, tag="x")
            rSb = (
                rS[:].rearrange("p (t o) -> p t o", o=1)
                .broadcast_to([H, T, HID])
            )
            nc.vector.tensor_tensor(
                x[:].rearrange("p (t c) -> p t c", c=HID),
                cr_ps[:].rearrange("p (t c) -> p t c", c=HID),
                rSb, op=OP.mult,
            )
            # ctx' = elu(x)+1 = relu(x) + min(exp(x), 1)
            ev = mid.tile([H, T * HID], bf, tag="ev")
            nc.scalar.activation(ev[:], x[:], AF.Exp)
            rn = mid.tile([H, T * HID], bf, tag="rn")
            nc.gpsimd.tensor_scalar_max(rn[:], x[:], 0.0)
            ctxn = mid.tile([H, T * HID], bf, tag="ctxn")
            nc.vector.scalar_tensor_tensor(
                ctxn[:], ev[:], 1.0, rn[:], OP.min, OP.add,
            )
            # transpose ctx to [HID, node] (ones row persistent in ctxT buf)
            ctxT_ps = pX.tile([HID, TH], bf, tag="ctxTp")
            for t in range(T):
                nc.tensor.transpose(
                    ctxT_ps[:, t * 128:(t + 1) * 128],
                    ctxn[:, t * HID:(t + 1) * HID],
                    identb,
                )
            ctxT = ctxTbufs[st["buf"]]
            nc.gpsimd.tensor_copy(ctxT[0:HID, 0:TH], ctxT_ps[:])
            st["ctxT"] = ctxT
            return st

        def stageC2(st):
            gi, T = st["gi"], st["T"]
            c0 = emit_off[gi] * 128
            TH = T * 128
            ctxT = st["ctxT"]

            # --- GRU gates ---
            t2 = mid.tile([H, TH], bf, tag="t2")
            sigq = mid.tile([H, 2 * TH], bf, tag="sigq")
            for pair in _pairs(T):
                P2 = len(pair)
                p0 = pair[0]
                rz_ps = pR.tile([H, P2 * 2 * H], f32, tag="rz")
                inhn_ps = pN.tile([H, P2 * 2 * H], f32, tag="inhn")
                for ti, t in enumerate(pair):
                    ctx_t = ctxT[:, t * H:(t + 1) * H]
                    nfT_c = nfT_all[:, c0 + t * H:c0 + (t + 1) * H]
                    nc.tensor.matmul(
                        rz_ps[:, ti * 2 * H:(ti + 1) * 2 * H],
                        ctx_t, wih[:, 0:2 * H], start=True, stop=False,
                    )
                    nc.tensor.matmul(
                        rz_ps[:, ti * 2 * H:(ti + 1) * 2 * H],
                        nfT_c, whh[:, 0:2 * H], start=False, stop=True,
                    )
                    # in_n = ctx@wih_n (+bias via ones row of ctxT)
                    nc.tensor.matmul(
                        inhn_ps[:, ti * 2 * H:ti * 2 * H + H],
                        ctx_t, wih[:, 2 * H:], start=True, stop=True,
                    )
                    # hn = 0.5*(nf@whh_n) + 0.5*bhh_n
                    nc.tensor.matmul(
                        inhn_ps[:, ti * 2 * H + H:(ti + 1) * 2 * H],
                        nfT_c, whh[:, 2 * H:], start=True, stop=False,
                    )
                    nc.tensor.matmul(
                        inhn_ps[:, ti * 2 * H + H:(ti + 1) * 2 * H],
                        ctxT[HID:HID + 1, t * H:(t + 1) * H], bhhn,
                        start=False, stop=True,
                    )
                # trz = tanh(0.5 * rz); r = (1+tr)/2, z = (1+tz)/2
                sig = sigq[:, p0 * 2 * H:(p0 + P2) * 2 * H]
                nc.scalar.activation(sig, rz_ps[:], AF.Tanh, scale=0.5)
                sigv = sig.rearrange("p (t two h) -> p t two h", two=2, h=H)
                tr_v = sigv[:, :, 0, :]
                iv = inhn_ps[:].rearrange("p (t two h) -> p t two h", two=2, h=H)
                in_v = iv[:, :, 0, :]
                hn_v = iv[:, :, 1, :]
                # t2 = (tr+1)*hn + in_n  (= i_n + r*(h_n + bhh_n))
                t1 = mid.tile([H, P2 * H], bf, tag="t1")
                nc.vector.scalar_tensor_tensor(
                    t1[:].rearrange("p (t h) -> p t h", h=H),
                    tr_v, 1.0, hn_v, OP.add, OP.mult,
                )
                nc.vector.tensor_tensor(
                    t2[:, p0 * H:(p0 + P2) * H].rearrange("p (t h) -> p t h", h=H),
                    t1[:].rearrange("p (t h) -> p t h", h=H),
                    in_v, op=OP.add,
                )
            # n = tanh(t2); h = relu(0.5*(n + nf + tz*(nf-n)))
            #   = relu(0.5*[(tz+1)*nf - (tz-1)*n])
            n_t = mid.tile([H, TH], bf, tag="n")
            nc.scalar.activation(n_t[:], t2[:], AF.Tanh)
            nf_v = nf_all[:, c0:c0 + TH]
            tz_q = sigq[:].rearrange("p (t two h) -> p t two h", two=2, h=H)[:, :, 1, :]
            u_t = mid.tile([H, TH], bf, tag="u")
            nc.vector.scalar_tensor_tensor(
                u_t[:].rearrange("p (t h) -> p t h", h=H),
                tz_q, 1.0,
                nf_v.rearrange("p (t h) -> p t h", h=H),
                OP.add, OP.mult,
            )
            v_t = mid.tile([H, TH], bf, tag="v")
            nc.vector.scalar_tensor_tensor(
                v_t[:].rearrange("p (t h) -> p t h", h=H),
                tz_q, -1.0, n_t[:].rearrange("p (t h) -> p t h", h=H),
                OP.add, OP.mult,
            )
            hp = mid.tile([H, TH], bf, tag="hp")
            nc.vector.tensor_tensor(hp[:], u_t[:], v_t[:], op=OP.subtract)
            nc.vector.tensor_scalar(
                ho_all[:, c0:c0 + TH], hp[:], 0.5, 0.0, op0=OP.mult, op1=OP.max,
            )

        # --- 4-stage software pipeline over groups:
        # A(g) dma | B(g-1) exp+y | C1(g-2) ctx | C2(g-3) gru ---
        NG = len(gspec)
        stA = stB = stC = None
        bufc = 0
        consumed = 0
        store1_cols = None
        for i in range(NG + 3):
            a = None
            if i < NG:
                with tc.high_priority(offset=1200):
                    a = stageA(order[i])
            if stA is not None:
                with tc.high_priority(offset=600):
                    newB = stageB(stA)
            else:
                newB = None
            if i == 1:
                # node features load after the first two pf groups
                nc.sync.dma_start(nfT_all[:], nfT_d)
                nc.sync.dma_start(nf_all[:], nf_d)
            if stB is not None:
                stB["buf"] = bufc % 2
                bufc += 1
                newC = stageC1(stB)
            else:
                newC = None
            if stC is not None:
                stageC2(stC)
                consumed += stC["T"]
            stA, stB, stC = a, newB, newC
            if i == NG + 1 and store1_cols is None:
                # partial store once all but the last two groups' tails are
                # queued; covers everything already computed
                store1_cols = consumed * 128
                nc.sync.dma_start(
                    hout_d[:, 0:store1_cols], ho_all[:, 0:store1_cols]
                )
        nc.sync.dma_start(
            hout_d[:, store1_cols:], ho_all[:, store1_cols:]
        )
        assert consumed == NT
